# revision 5
# baseline (speedup 1.0000x reference)
"""ConfidenceGuidedGate (MoE routing) Trainium2 kernel, v4.

Computes, for x [N=16384, D=4096], W [E=128, D], b [E]:
    logits = x @ W.T + b; conf = sigmoid(logits); top_k(conf, k=2)

Data-parallel over 8 NeuronCores (2048 tokens/core), W/b replicated.

v4 key change vs v3 (184.2us): x is transposed on the HOST per shard
(xt = x_shard.T, [D, N_TOK]), the same move v1-v3 already made for W.T.
That deletes the 512 PE transposes (~55us of tensor time at the 1.2 GHz
transpose-mode clock; HAM never boosts transpose-mode), the PSUM px
staging, and the scalar/vector reads-from-PSUM. The kernel becomes a
plain DMA-fed stream:
  - per (group g of 512 tokens, chunk c of 128 d): DMA xt[c*128:+128,
    g*512:+512] (2 KiB contiguous per partition line), scalar tf32
    rounding copy (xtr) + vector residual sub (xte) straight from SBUF,
    3 f32r matmuls accumulate logits.T in PSUM (tf32 x tf32 exact in
    fp32, same product/accumulation order as v3: top-k selection
    matches the fp32 reference on near-ties).
  - W.T split host-side into an exact tf32 pair (wtr + wtlo ~= W.T to
    2^-22), streamed as f32r piece-DMAs (4 chunks per piece) inter-
    leaved with group-0 x so the first matmul can start at ~2us.
  - epilogue per group (deferred into the next group's chunk loop):
    bias via scalar.activation, PE back-transpose, DVE Max8/MaxIndex
    top-2, sigmoid on the 2 winners; results collect in ONE partition-
    major SBUF tile written by a single 256B-per-partition DMA at the
    end (host undoes the layout).
  - PE warmup transposes at kernel start (HAM activity ramp) kept
    from v3.
Roofline: DMA 37.7 MB at ~358 GB/s/core ~= 105us; tensor ~90us.
"""

import numpy as np

import concourse.bass as bass
import concourse.mybir as mybir
from concourse.bass_utils import run_bass_kernel_spmd
from concourse.masks import make_identity
from concourse.tile import TileContext
from concourse.vector_clock import ScopedClock

N, D, E, K = 16384, 4096, 128, 2
N_CORES = 8
N_TOK = N // N_CORES          # 2048 tokens per core
P = 128
N_CHUNKS = D // P             # 32 contraction chunks
N_SLABS = N_TOK // P          # 16

F32 = mybir.dt.float32
F32R = mybir.dt.float32r
U32 = mybir.dt.uint32

GROUPS = [(0, 512), (512, 512), (1024, 512), (1536, 512)]
GW = 512                      # group width (tokens)
WPC = 4                       # weight chunks per weight DMA piece
N_WP = N_CHUNKS // WPC        # 8 weight pieces
PF = 28                       # x-DMA prefetch depth (chunk-group steps)
WARMUP_T = 16                 # dummy PE transposes to ramp HAM early

MAX_WAITS = 1


class PatchedTileContext(TileContext):
    """TileContext capping per-instruction sem waits to what walrus codegen
    accepts: excess waits hoisted onto same-engine NOPs (engines are
    in-order, so an earlier same-engine wait is semantics-preserving)."""

    _nop_ctr = 0

    def _add_instruction(self, inst):
        si = inst.sync_info
        if (
            si is not None
            and len(si.on_wait) > MAX_WAITS
            and inst.engine != mybir.EngineType.Unassigned
        ):
            waits = list(si.on_wait)
            keep = waits[:MAX_WAITS]
            rest = waits[MAX_WAITS:]
            while rest:
                PatchedTileContext._nop_ctr += 1
                nop = mybir.InstNoOp(
                    name=f"I-xw-{PatchedTileContext._nop_ctr}", ins=[], outs=[]
                )
                nop.engine = inst.engine
                nop.sync_info = mybir.SyncInfo(
                    on_wait=rest[:MAX_WAITS], on_update=[]
                )
                super()._add_instruction(nop)
                rest = rest[MAX_WAITS:]
            si.on_wait = keep
        super()._add_instruction(inst)

    def _drain_and_barrier(self, tick_clock, wait_clock):
        drain_inst = self.nc.sync.drain()
        wait_clock.add_sem_waits(
            drain_inst.ins, ScopedClock({None: tick_clock.global_clock})
        )
        mi = drain_inst.ins
        si = mi.sync_info
        if si is not None and len(si.on_wait) > MAX_WAITS:
            waits = list(si.on_wait)
            si.on_wait = waits[:MAX_WAITS]
            rest = waits[MAX_WAITS:]
            while rest:
                d2 = self.nc.sync.drain()
                d2.ins.sync_info = mybir.SyncInfo(
                    on_wait=rest[:MAX_WAITS], on_update=[]
                )
                rest = rest[MAX_WAITS:]
        self.nc.all_engine_barrier()
        assert self.sems is not None
        popped = self.nc._tile_sem_poison_stack.pop()
        assert popped is self._sem_poison
        self.nc.clear_and_free_semaphores(list(self.sems.allocated().values()))
        self.nc.all_engine_barrier()


def build_kernel() -> bass.Bass:
    nc = bass.Bass("TRN2", target_bir_lowering=False, debug=False)

    # host-transposed x shard: xt = x_shard.T, [D, N_TOK]
    xt_d = nc.declare_dram_parameter("xt", [D, N_TOK], F32, isOutput=False)
    # host-prepared exact tf32 split of W.T (f32r: values tf32-rounded on
    # the host; the BIR verifier requires f32r matmul inputs born as f32r)
    wtr_d = nc.declare_dram_parameter("wtr", [D, E], F32R, isOutput=False)
    wtlo_d = nc.declare_dram_parameter("wtlo", [D, E], F32R, isOutput=False)
    b_d = nc.declare_dram_parameter("b", [E], F32, isOutput=False)
    # combined output, partition-major: out[p, slab, 0:2]=top2 sigmoid vals,
    # out[p, slab, 2:4]=top2 indices (uint32 bits); token = slab*128 + p
    out_d = nc.declare_dram_parameter(
        "out", [P, N_SLABS * 2 * K], F32, isOutput=True
    )

    with PatchedTileContext(nc) as tc:
        with (
            tc.tile_pool(name="const", bufs=1) as const_pool,
            tc.tile_pool(name="wt", bufs=1) as wt_pool,
            tc.tile_pool(name="xt", bufs=PF + 2) as xt_pool,
            tc.tile_pool(name="xs", bufs=4) as xs_pool,
            tc.tile_pool(name="lsb", bufs=2) as lsb_pool,
            tc.tile_pool(name="top", bufs=6) as top_pool,
            tc.tile_pool(name="pt", bufs=2, space="PSUM") as psum_t,
            tc.tile_pool(name="pw", bufs=1, space="PSUM") as psum_w,
            tc.tile_pool(name="pl", bufs=2, space="PSUM") as psum_l,
        ):
            # PE warmup: dummy transposes keep the HAM activity window
            # busy while the first data DMAs land (dep: one gpsimd memset)
            wsrc = const_pool.tile([P, P], F32)
            nc.gpsimd.memset(wsrc[:], 1.0)
            warm = psum_w.tile([P, P], F32)
            for _ in range(WARMUP_T):
                nc.tensor.transpose(warm[:], wsrc[:], wsrc[:])

            ident = const_pool.tile([P, P], F32)
            make_identity(nc, ident[:])

            # combined output accumulator (written by every epilogue)
            out_sb = const_pool.tile([P, N_SLABS * 2 * K], F32)
            out_u32 = out_sb[:].bitcast(U32)

            # --- weights + group-0 x, interleaved so both ramp together -----
            wtr_all = wt_pool.tile([P, N_CHUNKS * P], F32R)
            wtlo_all = wt_pool.tile([P, N_CHUNKS * P], F32R)

            def issue_wt_piece(q):
                for dst, src in ((wtr_all, wtr_d), (wtlo_all, wtlo_d)):
                    nc.sync.dma_start(
                        out=dst[:]
                        .rearrange("p (c e) -> p c e", e=E)[
                            :, q * WPC : (q + 1) * WPC, :
                        ],
                        in_=src.rearrange("(c p) e -> p c e", p=P)[
                            :, q * WPC : (q + 1) * WPC, :
                        ],
                    )

            # linear step u = g*N_CHUNKS + c; x tile for step u
            x_tiles = {}
            x_issued = [0]  # next linear step whose DMA is not yet issued

            def issue_x(upto):
                while x_issued[0] < min(upto, len(GROUPS) * N_CHUNKS):
                    u = x_issued[0]
                    g, c = divmod(u, N_CHUNKS)
                    t0 = GROUPS[g][0]
                    t = xt_pool.tile([P, GW], F32, tag="xt")
                    nc.sync.dma_start(
                        out=t[:],
                        in_=xt_d[c * P : (c + 1) * P, t0 : t0 + GW],
                    )
                    x_tiles[u] = t
                    x_issued[0] = u + 1

            # ramp: weight piece q lands just before the chunks that need it
            # (mm chunk c needs piece c//WPC); x chunks fill the gaps
            issue_wt_piece(0)
            for q in range(1, N_WP):
                issue_x(WPC * (q - 1) + 2)
                issue_wt_piece(q)
            issue_x(PF)

            b_sb = const_pool.tile([P, 1], F32)
            nc.sync.dma_start(out=b_sb[:], in_=b_d[:])

            # --- epilogue (deferred into the next group's chunk loop) -------
            def run_epilogue(t0, ntok, pl):
                nslab = ntok // P
                lsb = lsb_pool.tile([P, GW], F32, tag="lsb")
                nc.scalar.activation(
                    lsb[:, :ntok], pl[:],
                    mybir.ActivationFunctionType.Identity,
                    bias=b_sb[:], scale=1.0,
                )
                ptb = psum_t.tile([P, ntok], F32, tag="ptb")
                for j in range(nslab):
                    nc.tensor.transpose(
                        ptb[:, j * P : (j + 1) * P],
                        lsb[:, j * P : (j + 1) * P],
                        ident[:],
                    )
                for j in range(nslab):
                    js = t0 // P + j  # global slab index
                    pv = ptb[:, j * P : (j + 1) * P]
                    vals8 = top_pool.tile([P, 8], F32, tag="v8")
                    idx8 = top_pool.tile([P, 8], U32, tag="i8")
                    nc.vector.max(vals8[:], pv)
                    nc.vector.max_index(idx8[:], vals8[:], pv)
                    nc.scalar.activation(
                        out_sb[:, js * 4 : js * 4 + K],
                        vals8[:, :K],
                        mybir.ActivationFunctionType.Sigmoid,
                    )
                    nc.vector.tensor_copy(
                        out_u32[:, js * 4 + K : js * 4 + 2 * K], idx8[:, :K]
                    )

            # --- main pipeline ----------------------------------------------
            deferred = None
            for g, (t0, ntok) in enumerate(GROUPS):
                pl = psum_l.tile([P, ntok], F32, tag="pl")
                for c in range(N_CHUNKS):
                    u = g * N_CHUNKS + c
                    if c == 4 and deferred is not None:
                        run_epilogue(*deferred)
                        deferred = None
                    issue_x(u + 1 + PF)
                    xt = x_tiles.pop(u)
                    xtr = xs_pool.tile([P, GW], F32R, tag="xtr")
                    xte = xs_pool.tile([P, GW], F32R, tag="xte")
                    nc.scalar.copy(xtr[:], xt[:])
                    nc.vector.tensor_sub(xte[:], xt[:], xtr[:].bitcast(F32))
                    wr = wtr_all[:, c * P : (c + 1) * P]
                    wl = wtlo_all[:, c * P : (c + 1) * P]
                    nc.tensor.matmul(
                        pl[:], wr, xtr[:], start=(c == 0), stop=False
                    )
                    nc.tensor.matmul(
                        pl[:], wr, xte[:], start=False, stop=False
                    )
                    nc.tensor.matmul(
                        pl[:], wl, xtr[:],
                        start=False, stop=(c == N_CHUNKS - 1),
                    )

                if g == len(GROUPS) - 1:
                    run_epilogue(t0, ntok, pl)
                else:
                    deferred = (t0, ntok, pl)

            # single dense output DMA: 256B contiguous per partition
            nc.sync.dma_start(out=out_d[:, :], in_=out_sb[:])

    return nc


_NC_CACHE = None


def _get_nc():
    global _NC_CACHE
    if _NC_CACHE is None:
        _NC_CACHE = build_kernel()
    return _NC_CACHE


def _rne_tf32(a: np.ndarray) -> np.ndarray:
    """Round fp32 to tf32 (11-bit mantissa), round-to-nearest-even."""
    bits = a.view(np.uint32).astype(np.uint64)
    lsb = (bits >> 13) & 1
    bits = bits + 0x0FFF + lsb
    bits = (bits & np.uint64(0xFFFFE000)).astype(np.uint32)
    return bits.view(np.float32)


def prep_weights(W: np.ndarray):
    """Host-side layout + exact-product tf32 split of the gate weight."""
    WT = np.ascontiguousarray(W.T.astype(np.float32))
    wtr = _rne_tf32(WT)
    wtlo = _rne_tf32(WT - wtr)
    return wtr, wtlo


def _unpack(out: np.ndarray):
    """[P, 16*4] partition-major -> (vals [2048,2] f32, idx [2048,2] i32)."""
    a = out.reshape(P, N_SLABS, 2 * K).transpose(1, 0, 2).reshape(N_TOK, 2 * K)
    vals = a[:, :K].copy()
    idx = a[:, K : 2 * K].copy().view(np.int32)
    return vals, idx


def run_sharded(x, wtr, wtlo, b, trace=False, **kw):
    nc = _get_nc()
    in_maps = []
    for i in range(N_CORES):
        in_maps.append(
            {
                "xt": np.ascontiguousarray(
                    x[i * N_TOK : (i + 1) * N_TOK].T
                ),
                "wtr": wtr,
                "wtlo": wtlo,
                "b": b,
            }
        )
    return run_bass_kernel_spmd(
        nc, in_maps, core_ids=list(range(N_CORES)), trace=trace, **kw
    )


def kernel(x, W, b):
    x = np.asarray(x, dtype=np.float32)
    W = np.asarray(W, dtype=np.float32)
    b = np.asarray(b, dtype=np.float32)
    wtr, wtlo = prep_weights(W)
    res = run_sharded(x, wtr, wtlo, b)
    vals_l, idx_l = [], []
    for r in res.results:
        v, i = _unpack(r["out"])
        vals_l.append(v)
        idx_l.append(i)
    return np.concatenate(vals_l, axis=0), np.concatenate(idx_l, axis=0)


# revision 9
# speedup vs baseline: 1.1245x; 1.1245x over previous
"""ConfidenceGuidedGate (MoE routing) Trainium2 kernel, v7.

Computes, for x [N=16384, D=4096], W [E=128, D], b [E]:
    logits = x @ W.T + b; conf = sigmoid(logits); top_k(conf, k=2)

Data-parallel over 8 NeuronCores (2048 tokens/core), W/b replicated.

Lineage: v3 184.2us (PE-transposed x) -> v4/v5 135.4us (host-transposed
x, 256KB chunk DMAs) -> v7. v5 trace taught:
  - DMA efficiency scales with transfer size (341 GB/s at 1 MB vs ~300
    at 256 KB), so x now streams as 1 MB super-DMAs covering 4
    contraction chunks of one token group each (3D access pattern,
    2 KiB runs).
  - TileContext teardown pays ~0.4us per pool buffer (serial 4-engine
    token loop at exit): pools use few, large buffers.
  - weight DMA bytes displace x bytes on the critical path: W.T now
    ships as ONE 2 MB f32 tensor and the exact tf32 split (wtr + wtlo)
    is computed on-chip per 4-chunk piece (scalar f32r rounding copy +
    vector residual sub, the same proven ops the x split uses). The
    split ops for pieces 1..7 sit INSIDE group 0's chunk loop so the
    in-order scalar/vector queues never head-of-line block x splits.
  - scalar/vector tf32-split ops run 1024 wide (2 chunks per op) to
    halve instruction overhead.
Structure: 4 token groups of 512 (PSUM logits.T bank each), group-outer
chunk-inner loop, 3 f32r matmuls per (group, chunk) accumulating
tf32-exact products (top-k selection matches fp32 reference on
near-ties). Epilogue per group (bias activation, PE back-transpose,
DVE Max8/MaxIndex top-2, sigmoid) is deferred into the next group's
loop; one 256B-per-partition output DMA at the end.
"""

import numpy as np

import concourse.bass as bass
import concourse.mybir as mybir
from concourse.bass_utils import run_bass_kernel_spmd
from concourse.masks import make_identity
from concourse.tile import TileContext
from concourse.vector_clock import ScopedClock

N, D, E, K = 16384, 4096, 128, 2
N_CORES = 8
N_TOK = N // N_CORES          # 2048 tokens per core
P = 128
N_CHUNKS = D // P             # 32 contraction chunks
N_SLABS = N_TOK // P          # 16

F32 = mybir.dt.float32
F32R = mybir.dt.float32r
U32 = mybir.dt.uint32

GROUPS = [(0, 512), (512, 512), (1024, 512), (1536, 512)]
GW = 512                      # group width (tokens)
# super-DMA chunk counts per group: first two small to shorten the ramp
XPAT = [2, 2, 4, 4, 4, 4, 4, 4, 4]
SW = 2                        # chunks per split op (1024-wide scalar/vector)
WPC = 4                       # weight chunks per piece (DMA + split)
N_WP = N_CHUNKS // WPC        # 8 weight pieces
XPF = 4                       # super-DMA prefetch depth
WARMUP_T = 16                 # dummy PE transposes to ramp HAM early

MAX_WAITS = 1


class PatchedTileContext(TileContext):
    """TileContext capping per-instruction sem waits to what walrus codegen
    accepts: excess waits hoisted onto same-engine NOPs (engines are
    in-order, so an earlier same-engine wait is semantics-preserving)."""

    _nop_ctr = 0

    def _add_instruction(self, inst):
        si = inst.sync_info
        if (
            si is not None
            and len(si.on_wait) > MAX_WAITS
            and inst.engine != mybir.EngineType.Unassigned
        ):
            waits = list(si.on_wait)
            keep = waits[:MAX_WAITS]
            rest = waits[MAX_WAITS:]
            while rest:
                PatchedTileContext._nop_ctr += 1
                nop = mybir.InstNoOp(
                    name=f"I-xw-{PatchedTileContext._nop_ctr}", ins=[], outs=[]
                )
                nop.engine = inst.engine
                nop.sync_info = mybir.SyncInfo(
                    on_wait=rest[:MAX_WAITS], on_update=[]
                )
                super()._add_instruction(nop)
                rest = rest[MAX_WAITS:]
            si.on_wait = keep
        super()._add_instruction(inst)

    def _drain_and_barrier(self, tick_clock, wait_clock):
        drain_inst = self.nc.sync.drain()
        wait_clock.add_sem_waits(
            drain_inst.ins, ScopedClock({None: tick_clock.global_clock})
        )
        mi = drain_inst.ins
        si = mi.sync_info
        if si is not None and len(si.on_wait) > MAX_WAITS:
            waits = list(si.on_wait)
            si.on_wait = waits[:MAX_WAITS]
            rest = waits[MAX_WAITS:]
            while rest:
                d2 = self.nc.sync.drain()
                d2.ins.sync_info = mybir.SyncInfo(
                    on_wait=rest[:MAX_WAITS], on_update=[]
                )
                rest = rest[MAX_WAITS:]
        self.nc.all_engine_barrier()
        assert self.sems is not None
        popped = self.nc._tile_sem_poison_stack.pop()
        assert popped is self._sem_poison
        self.nc.clear_and_free_semaphores(list(self.sems.allocated().values()))
        self.nc.all_engine_barrier()


def build_kernel() -> bass.Bass:
    nc = bass.Bass("TRN2", target_bir_lowering=False, debug=False)

    # host-transposed x shard: xt = x_shard.T, [D, N_TOK]
    xt_d = nc.declare_dram_parameter("xt", [D, N_TOK], F32, isOutput=False)
    # host-prepared W.T (plain f32; exact tf32 split happens on-chip)
    wt_d = nc.declare_dram_parameter("wt", [D, E], F32, isOutput=False)
    b_d = nc.declare_dram_parameter("b", [E], F32, isOutput=False)
    # combined output, partition-major: out[p, slab, 0:2]=top2 sigmoid vals,
    # out[p, slab, 2:4]=top2 indices (uint32 bits); token = slab*128 + p
    out_d = nc.declare_dram_parameter(
        "out", [P, N_SLABS * 2 * K], F32, isOutput=True
    )

    with PatchedTileContext(nc) as tc:
        with (
            tc.tile_pool(name="const", bufs=1) as const_pool,
            tc.tile_pool(name="wt", bufs=1) as wt_pool,
            tc.tile_pool(name="xt", bufs=5) as xt_pool,
            tc.tile_pool(name="xs", bufs=6) as xs_pool,
            tc.tile_pool(name="lsb", bufs=2) as lsb_pool,
            tc.tile_pool(name="top", bufs=6) as top_pool,
            tc.tile_pool(name="pt", bufs=2, space="PSUM") as psum_t,
            tc.tile_pool(name="pw", bufs=1, space="PSUM") as psum_w,
            tc.tile_pool(name="pl", bufs=2, space="PSUM") as psum_l,
        ):
            # PE warmup: dummy transposes keep the HAM activity window
            # busy while the first data DMAs land (dep: one gpsimd memset)
            wsrc = const_pool.tile([P, P], F32)
            nc.gpsimd.memset(wsrc[:], 1.0)
            warm = psum_w.tile([P, P], F32)
            for _ in range(WARMUP_T):
                nc.tensor.transpose(warm[:], wsrc[:], wsrc[:])

            ident = const_pool.tile([P, P], F32)
            make_identity(nc, ident[:])

            # combined output accumulator (written by every epilogue)
            out_sb = const_pool.tile([P, N_SLABS * 2 * K], F32)
            out_u32 = out_sb[:].bitcast(U32)

            # --- weights: one f32 stream, split to tf32 pair on-chip -------
            wt_sb = wt_pool.tile([P, N_CHUNKS * E], F32)
            wtr_all = wt_pool.tile([P, N_CHUNKS * E], F32R)
            wtlo_all = wt_pool.tile([P, N_CHUNKS * E], F32R)

            def issue_wt_dma(q):
                nc.sync.dma_start(
                    out=wt_sb[:]
                    .rearrange("p (c e) -> p c e", e=E)[
                        :, q * WPC : (q + 1) * WPC, :
                    ],
                    in_=wt_d.rearrange("(c p) e -> p c e", p=P)[
                        :, q * WPC : (q + 1) * WPC, :
                    ],
                )

            def split_wt_piece(q):
                s = slice(q * WPC * E, (q + 1) * WPC * E)
                nc.scalar.copy(wtr_all[:, s], wt_sb[:, s])
                nc.vector.tensor_sub(
                    wtlo_all[:, s], wt_sb[:, s], wtr_all[:, s].bitcast(F32)
                )

            # x super-DMAs: linear list of (group, chunk_lo, n_chunks)
            xplan = []
            xidx = {}  # (g, chunk) -> linear super-tile index
            for g, (t0, _) in enumerate(GROUPS):
                c0 = 0
                for nch in XPAT:
                    for c in range(c0, c0 + nch):
                        xidx[(g, c)] = len(xplan)
                    xplan.append((g, c0, nch))
                    c0 += nch
                assert c0 == N_CHUNKS
            # chunk -> (tile, col offset) map, filled as DMAs are issued
            x_tiles = {}
            x_issued = [0]

            def issue_x(upto):
                while x_issued[0] < min(upto, len(xplan)):
                    g, c0, nch = xplan[x_issued[0]]
                    t0 = GROUPS[g][0]
                    t = xt_pool.tile([P, nch * GW], F32, tag="xt")
                    nc.sync.dma_start(
                        out=t[:].rearrange("p (c t) -> p c t", t=GW),
                        in_=xt_d.rearrange("(c p) t -> p c t", p=P)[
                            :, c0 : c0 + nch, t0 : t0 + GW
                        ],
                    )
                    for c in range(c0, c0 + nch):
                        x_tiles[(g, c)] = (t, (c - c0) * GW)
                    x_issued[0] += 1

            # ramp: weight pieces land just ahead of the chunks needing them
            issue_wt_dma(0)
            issue_wt_dma(1)
            issue_x(1)
            issue_wt_dma(2)
            issue_wt_dma(3)
            issue_x(2)
            issue_wt_dma(4)
            issue_wt_dma(5)
            issue_x(3)
            issue_wt_dma(6)
            issue_wt_dma(7)
            issue_x(XPF)

            b_sb = const_pool.tile([P, 1], F32)
            nc.sync.dma_start(out=b_sb[:], in_=b_d[:])

            split_wt_piece(0)

            # --- epilogue (deferred into the next group's chunk loop) -------
            def run_epilogue(t0, ntok, pl):
                nslab = ntok // P
                lsb = lsb_pool.tile([P, GW], F32, tag="lsb")
                nc.scalar.activation(
                    lsb[:, :ntok], pl[:],
                    mybir.ActivationFunctionType.Identity,
                    bias=b_sb[:], scale=1.0,
                )
                ptb = psum_t.tile([P, ntok], F32, tag="ptb")
                for j in range(nslab):
                    nc.tensor.transpose(
                        ptb[:, j * P : (j + 1) * P],
                        lsb[:, j * P : (j + 1) * P],
                        ident[:],
                    )
                for j in range(nslab):
                    js = t0 // P + j  # global slab index
                    pv = ptb[:, j * P : (j + 1) * P]
                    vals8 = top_pool.tile([P, 8], F32, tag="v8")
                    idx8 = top_pool.tile([P, 8], U32, tag="i8")
                    nc.vector.max(vals8[:], pv)
                    nc.vector.max_index(idx8[:], vals8[:], pv)
                    nc.scalar.activation(
                        out_sb[:, js * 4 : js * 4 + K],
                        vals8[:, :K],
                        mybir.ActivationFunctionType.Sigmoid,
                    )
                    nc.vector.tensor_copy(
                        out_u32[:, js * 4 + K : js * 4 + 2 * K], idx8[:, :K]
                    )

            # --- main pipeline ----------------------------------------------
            deferred = None
            xu = 0  # linear super-DMA consumption counter
            for g, (t0, ntok) in enumerate(GROUPS):
                pl = psum_l.tile([P, ntok], F32, tag="pl")
                xts = {}
                for c in range(N_CHUNKS):
                    if c == 4 and deferred is not None:
                        run_epilogue(*deferred)
                        deferred = None
                    # on-chip W split for pieces 1..7, inside group 0's loop
                    # (their wt DMAs landed during the ramp; placing the ops
                    # here keeps them from head-of-line blocking x splits)
                    if g == 0 and c >= 2 and c % WPC == 2 and c // WPC < N_WP - 1:
                        split_wt_piece(c // WPC + 1)
                    if c % SW == 0:
                        # 1024-wide tf32 split covering chunks c, c+1
                        tile, off = x_tiles.pop((g, c))
                        tile2, off2 = x_tiles.pop((g, c + 1))
                        assert tile2 is tile and off2 == off + GW
                        xv = tile[:, off : off + SW * GW]
                        xtr = xs_pool.tile([P, SW * GW], F32R, tag="xtr")
                        xte = xs_pool.tile([P, SW * GW], F32R, tag="xte")
                        nc.scalar.copy(xtr[:], xv)
                        nc.vector.tensor_sub(xte[:], xv, xtr[:].bitcast(F32))
                        xts[c] = (xtr, 0)
                        xts[c + 1] = (xtr, GW)
                        xts[(c, "e")] = (xte, 0)
                        xts[(c + 1, "e")] = (xte, GW)
                        # keep the super-DMA stream XPF tiles ahead of use
                        issue_x(xidx[(g, c)] + 1 + XPF)
                    xtr, xo = xts[c]
                    xte, _ = xts[(c, "e")]
                    wr = wtr_all[:, c * P : (c + 1) * P]
                    wl = wtlo_all[:, c * P : (c + 1) * P]
                    nc.tensor.matmul(
                        pl[:], wr, xtr[:, xo : xo + GW],
                        start=(c == 0), stop=False,
                    )
                    nc.tensor.matmul(
                        pl[:], wr, xte[:, xo : xo + GW], start=False, stop=False
                    )
                    nc.tensor.matmul(
                        pl[:], wl, xtr[:, xo : xo + GW],
                        start=False, stop=(c == N_CHUNKS - 1),
                    )

                if g == len(GROUPS) - 1:
                    run_epilogue(t0, ntok, pl)
                else:
                    deferred = (t0, ntok, pl)

            # single dense output DMA: 256B contiguous per partition
            nc.sync.dma_start(out=out_d[:, :], in_=out_sb[:])

    return nc


_NC_CACHE = None


def _get_nc():
    global _NC_CACHE
    if _NC_CACHE is None:
        _NC_CACHE = build_kernel()
    return _NC_CACHE


def _unpack(out: np.ndarray):
    """[P, 16*4] partition-major -> (vals [2048,2] f32, idx [2048,2] i32)."""
    a = out.reshape(P, N_SLABS, 2 * K).transpose(1, 0, 2).reshape(N_TOK, 2 * K)
    vals = a[:, :K].copy()
    idx = a[:, K : 2 * K].copy().view(np.int32)
    return vals, idx


def run_sharded(x, WT, b, trace=False, **kw):
    nc = _get_nc()
    in_maps = []
    for i in range(N_CORES):
        in_maps.append(
            {
                "xt": np.ascontiguousarray(
                    x[i * N_TOK : (i + 1) * N_TOK].T
                ),
                "wt": WT,
                "b": b,
            }
        )
    return run_bass_kernel_spmd(
        nc, in_maps, core_ids=list(range(N_CORES)), trace=trace, **kw
    )


def kernel(x, W, b):
    x = np.asarray(x, dtype=np.float32)
    W = np.asarray(W, dtype=np.float32)
    b = np.asarray(b, dtype=np.float32)
    WT = np.ascontiguousarray(W.T)
    res = run_sharded(x, WT, b)
    vals_l, idx_l = [], []
    for r in res.results:
        v, i = _unpack(r["out"])
        vals_l.append(v)
        idx_l.append(i)
    return np.concatenate(vals_l, axis=0), np.concatenate(idx_l, axis=0)
